# revision 1
# baseline (speedup 1.0000x reference)
"""ConvLSTM2D with per-sample expert routing (MoE) on 8 Trainium2 NeuronCores.

Data-parallel over batch: one sample per core. Everything below is hardcoded
for the problem shapes:
  inputs (8, 10, 64, 64, 32) fp32, label_values (8,) int32,
  kernel (3, 3, 32, 256), recurrent_kernel (4, 3, 3, 64, 256), bias (4, 256).

Per core, per timestep, the 3x3 convs are computed as 9 accumulating matmuls
into PSUM using row-shifted channel-stacked replicas of the padded images:
  - input conv:  x replicated 3x (row shifts 0,1,2) -> K=96, 3 matmuls (one
    per kernel column j), built host-side and streamed per step.
  - recurrent conv: h kept in SBUF as 2 row-shifted copies (shifts 0,1) on
    partitions 0-63 / 64-127 -> 3 matmuls with K=128 (rows dy=0,1) plus 3
    matmuls with K=64 on partitions 64-127 (row dy=2).
Matmuls run in float32r (full-rate fp32 with reduced mantissa).
Gate order is the natural [i, f, g, o]: PSUM chunk A = [i; f], chunk B = [g; o].
"""

import numpy as np

import concourse.bacc as bacc
import concourse.mybir as mybir
import concourse.tile as tile
from concourse.bass_utils import run_bass_kernel_spmd

F32 = mybir.dt.float32
F32R = mybir.dt.float32r
AF = mybir.ActivationFunctionType

B, T, H, W, CIN = 8, 10, 64, 64, 32
F = 64
NPIX = H * W                 # 4096
PH, PW = 66, 66              # padded image in SBUF
SBLK = 4                     # super blocks per step
SBH = 16                     # image rows per super block
NB = SBH * W                 # 1024 free elems per psum tile
MMR = 8                      # image rows per matmul (N = 512)

_CACHE = {}


def _build():
    nc = bacc.Bacc("TRN2", target_bir_lowering=False, debug=True)

    xq_d = nc.declare_dram_parameter("xq", [T, 96, PH, PW], F32, isOutput=False)
    win_d = nc.declare_dram_parameter("w_in", [96, 3, 256], F32, isOutput=False)
    u_d = nc.declare_dram_parameter("u", [128, 3, 256], F32, isOutput=False)
    u2_d = nc.declare_dram_parameter("u2", [64, 3, 256], F32, isOutput=False)
    bias_d = nc.declare_dram_parameter("bias2", [128, 2], F32, isOutput=False)
    hz_d = nc.declare_dram_parameter("hz", [128, PH * PW], F32, isOutput=False)
    y_d = nc.declare_dram_parameter("y", [T, F, NPIX], F32, isOutput=True)

    with tile.TileContext(nc) as tc:
        with tc.tile_pool(name="consts", bufs=1) as consts, \
             tc.tile_pool(name="state", bufs=1) as state, \
             tc.tile_pool(name="xqp", bufs=2) as xqp, \
             tc.tile_pool(name="psum", bufs=2, space="PSUM") as psp, \
             tc.tile_pool(name="gates", bufs=2) as gp:

            # ---- constants ----
            win_t = consts.tile([96, 3, 256], F32R)
            nc.sync.dma_start(out=win_t, in_=win_d[:].bitcast(F32R))
            u_t = consts.tile([128, 3, 256], F32R)
            nc.sync.dma_start(out=u_t, in_=u_d[:].bitcast(F32R))
            u2_t = consts.tile([128, 3, 256], F32R)
            nc.sync.dma_start(out=u2_t[64:128, :, :], in_=u2_d[:].bitcast(F32R))
            bias_t = consts.tile([128, 2], F32)
            nc.sync.dma_start(out=bias_t, in_=bias_d[:])

            # ---- state (ping-pong) ----
            hz3 = hz_d[:].rearrange("p (a b) -> p a b", a=PH)
            hbufs = []
            for i in range(2):
                hb = state.tile([128, PH, PW], F32R, tag=f"hb{i}")
                nc.sync.dma_start(out=hb, in_=hz3.bitcast(F32R))
                hbufs.append(hb)
            cbufs = []
            for i in range(2):
                cb = state.tile([128, NPIX], F32, tag=f"cb{i}")
                cbufs.append(cb)
            # only c[t=0] needs zero-init; cb1 is fully written at t=0
            nc.sync.dma_start(out=cbufs[0][64:128, :], in_=hz_d[64:128, 0:NPIX])

            for t in range(T):
                hcur = hbufs[t % 2]
                hnxt = hbufs[(t + 1) % 2]
                ccur = cbufs[t % 2]
                cnxt = cbufs[(t + 1) % 2]

                xq_t = xqp.tile([96, PH, PW], F32R, tag="xq")
                nc.sync.dma_start(out=xq_t, in_=xq_d[t].bitcast(F32R))

                for sb in range(SBLK):
                    r0 = SBH * sb
                    zA = psp.tile([128, NB], F32, tag="zA")
                    zB = psp.tile([128, NB], F32, tag="zB")
                    for m, z in ((0, zA), (1, zB)):
                        mc = slice(128 * m, 128 * m + 128)
                        for half in range(2):
                            rr = r0 + MMR * half
                            out = z[:, 512 * half:512 * half + 512]
                            for j in range(3):
                                nc.tensor.matmul(
                                    out, win_t[:, j, mc],
                                    xq_t[:, rr:rr + MMR, j:j + W],
                                    start=(j == 0), stop=False)
                            for j in range(3):
                                nc.tensor.matmul(
                                    out, u_t[:, j, mc],
                                    hcur[:, rr:rr + MMR, j:j + W],
                                    start=False, stop=False)
                            for j in range(3):
                                nc.tensor.matmul(
                                    out, u2_t[64:128, j, mc],
                                    hcur[64:128, rr + 1:rr + MMR + 1, j:j + W],
                                    start=False, stop=(j == 2))

                    blk = slice(NB * sb, NB * sb + NB)
                    # gates:  zA = [i; f] (both sigmoid), zB = [g; o]
                    pif = gp.tile([128, NB], F32, tag="pif")
                    nc.scalar.activation(out=pif, in_=zA[:], func=AF.Sigmoid,
                                         bias=bias_t[:, 0:1], scale=1.0)
                    S = gp.tile([128, NB], F32, tag="S")      # [g ; c]
                    nc.scalar.activation(out=S[0:64, :], in_=zB[0:64, :],
                                         func=AF.Tanh,
                                         bias=bias_t[0:64, 1:2], scale=1.0)
                    ot = gp.tile([128, NB], F32, tag="ot")    # [- ; o]
                    nc.scalar.activation(out=ot[64:128, :], in_=zB[64:128, :],
                                         func=AF.Sigmoid,
                                         bias=bias_t[64:128, 1:2], scale=1.0)
                    nc.vector.tensor_copy(S[64:128, :], ccur[64:128, blk])
                    R = gp.tile([128, NB], F32, tag="R")      # [i*g ; f*c]
                    nc.vector.tensor_mul(R, pif, S)
                    R2 = gp.tile([128, NB], F32, tag="R2")    # [- ; i*g]
                    nc.gpsimd.tensor_copy(out=R2[64:128, :], in_=R[0:64, :])
                    nc.vector.tensor_add(cnxt[64:128, blk], R2[64:128, :],
                                         R[64:128, :])
                    tct = gp.tile([128, NB], F32, tag="tct")  # [- ; tanh(c')]
                    nc.scalar.activation(out=tct[64:128, :],
                                         in_=cnxt[64:128, blk],
                                         func=AF.Tanh, scale=1.0)
                    # h = o * tanh(c') straight into the shifted copy (q=1)
                    nc.vector.tensor_mul(
                        hnxt[64:128, r0:r0 + SBH, 1:65],
                        ot[64:128, :].rearrange("p (a b) -> p a b", a=SBH),
                        tct[64:128, :].rearrange("p (a b) -> p a b", a=SBH))
                    # replicate into the unshifted copy (q=0), crossing lanes
                    nc.gpsimd.tensor_copy(
                        out=hnxt[0:64, r0 + 1:r0 + SBH + 1, 1:65],
                        in_=hnxt[64:128, r0:r0 + SBH, 1:65])
                    nc.sync.dma_start(
                        out=y_d[t, :, NB * sb:NB * sb + NB],
                        in_=hnxt[64:128, r0:r0 + SBH, 1:65].bitcast(F32))
    nc.compile()
    return nc


def _prep_inputs(inputs, label_values, kernel, recurrent_kernel, bias):
    x = np.ascontiguousarray(np.asarray(inputs, dtype=np.float32))
    lv = np.asarray(label_values).astype(np.int64)
    k = np.asarray(kernel, dtype=np.float32)
    rk_all = np.asarray(recurrent_kernel, dtype=np.float32)
    bias_all = np.asarray(bias, dtype=np.float32)

    x_cm = x.transpose(0, 1, 4, 2, 3)                      # (B, T, 32, 64, 64)
    pad = np.zeros((B, T, CIN, 68, PW), np.float32)
    pad[:, :, :, 1:65, 1:65] = x_cm
    xq = np.empty((B, T, 96, PH, PW), np.float32)
    for q in range(3):
        xq[:, :, 32 * q:32 * q + 32] = pad[:, :, :, q:q + PH, :]

    w_in = np.empty((96, 3, 256), np.float32)
    for q in range(3):
        w_in[32 * q:32 * q + 32] = k[q].transpose(1, 0, 2)  # (32, 3, 256)

    rk = rk_all[lv]                                         # (B, 3, 3, 64, 256)
    u = np.empty((B, 128, 3, 256), np.float32)
    for q in range(2):
        u[:, 64 * q:64 * q + 64] = rk[:, q].transpose(0, 2, 1, 3)
    u2 = np.ascontiguousarray(rk[:, 2].transpose(0, 2, 1, 3))  # (B, 64, 3, 256)

    bias_b = bias_all[lv]                                   # (B, 256)
    bias2 = np.ascontiguousarray(
        bias_b.reshape(B, 2, 128).transpose(0, 2, 1))       # (B, 128, 2)

    hz = np.zeros((128, PH * PW), np.float32)
    in_maps = []
    for b in range(B):
        in_maps.append({
            "xq": np.ascontiguousarray(xq[b]),
            "w_in": w_in,
            "u": np.ascontiguousarray(u[b]),
            "u2": u2[b],
            "bias2": bias2[b],
            "hz": hz,
        })
    return in_maps


def _run(inputs, label_values, kernel, recurrent_kernel, bias, **spmd_kwargs):
    if "nc" not in _CACHE:
        _CACHE["nc"] = _build()
    nc = _CACHE["nc"]
    in_maps = _prep_inputs(inputs, label_values, kernel, recurrent_kernel, bias)
    res = run_bass_kernel_spmd(nc, in_maps, list(range(B)), **spmd_kwargs)
    y = np.stack([res.results[b]["y"] for b in range(B)])   # (B, T, 64, 4096)
    out = y.reshape(B, T, F, H, W).transpose(0, 1, 3, 4, 2)
    return np.ascontiguousarray(out.astype(np.float32)), res


def kernel(inputs, label_values, kernel, recurrent_kernel, bias):
    out, _ = _run(inputs, label_values, kernel, recurrent_kernel, bias)
    return out


# revision 2
# speedup vs baseline: 1.0050x; 1.0050x over previous
"""ConvLSTM2D with per-sample expert routing (MoE) on 8 Trainium2 NeuronCores.

Data-parallel over batch: one sample per core. Everything below is hardcoded
for the problem shapes:
  inputs (8, 10, 64, 64, 32) fp32, label_values (8,) int32,
  kernel (3, 3, 32, 256), recurrent_kernel (4, 3, 3, 64, 256), bias (4, 256).

Per core, per timestep, the 3x3 convs are computed as 9 accumulating matmuls
into PSUM using row-shifted channel-stacked replicas of the padded images:
  - input conv:  x replicated 3x (row shifts 0,1,2) -> K=96, 3 matmuls (one
    per kernel column j), built host-side and streamed per step.
  - recurrent conv: h kept in SBUF as 2 row-shifted copies (shifts 0,1) on
    partitions 0-63 / 64-127 -> 3 matmuls with K=128 (rows dy=0,1) plus 3
    matmuls with K=64 on partitions 64-127 (row dy=2).
Gate order is the natural [i, f, g, o]: PSUM chunk A = [i; f], chunk B = [g; o].
Gate math stays fp32 (PSUM accumulation, ACT sigmoid/tanh with fused bias,
DVE products); matmul operands are MM_DT (bf16 by default, float32r option).
"""

import numpy as np
import ml_dtypes

import concourse.bacc as bacc
import concourse.mybir as mybir
import concourse.tile as tile
from concourse.bass_utils import run_bass_kernel_spmd

F32 = mybir.dt.float32
F32R = mybir.dt.float32r
BF16 = mybir.dt.bfloat16
AF = mybir.ActivationFunctionType

MM_DT = "bf16"          # "bf16" | "f32r"

B, T, H, W, CIN = 8, 10, 64, 64, 32
F = 64
NPIX = H * W                 # 4096
PH, PW = 66, 66              # padded image in SBUF
SBLK = 4                     # super blocks per step
SBH = 16                     # image rows per super block
NB = SBH * W                 # 1024 free elems per psum tile
MMR = 8                      # image rows per matmul (N = 512)

_CACHE = {}


def _build():
    mm_dt = BF16 if MM_DT == "bf16" else F32R
    io_dt = BF16 if MM_DT == "bf16" else F32   # dram dtype of matmul operands

    def mm_view(dram_ap):
        return dram_ap if MM_DT == "bf16" else dram_ap.bitcast(F32R)

    nc = bacc.Bacc("TRN2", target_bir_lowering=False, debug=True)

    xq_d = nc.declare_dram_parameter("xq", [T, 96, PH, PW], io_dt, isOutput=False)
    win_d = nc.declare_dram_parameter("w_in", [96, 3, 256], io_dt, isOutput=False)
    u_d = nc.declare_dram_parameter("u", [128, 3, 256], io_dt, isOutput=False)
    u2_d = nc.declare_dram_parameter("u2", [64, 3, 256], io_dt, isOutput=False)
    bias_d = nc.declare_dram_parameter("bias2", [128, 2], F32, isOutput=False)
    hz_d = nc.declare_dram_parameter("hz", [128, PH * PW], io_dt, isOutput=False)
    cz_d = nc.declare_dram_parameter("cz", [128, NPIX], F32, isOutput=False)
    y_d = nc.declare_dram_parameter("y", [T, F, NPIX], F32, isOutput=True)

    with tile.TileContext(nc) as tc:
        with tc.tile_pool(name="consts", bufs=1) as consts, \
             tc.tile_pool(name="state", bufs=1) as state, \
             tc.tile_pool(name="xqp", bufs=2) as xqp, \
             tc.tile_pool(name="psum", bufs=2, space="PSUM") as psp, \
             tc.tile_pool(name="gates", bufs=2) as gp:

            # ---- constants ----
            win_t = consts.tile([96, 3, 256], mm_dt)
            nc.sync.dma_start(out=win_t, in_=mm_view(win_d[:]))
            u_t = consts.tile([128, 3, 256], mm_dt)
            nc.sync.dma_start(out=u_t, in_=mm_view(u_d[:]))
            u2_t = consts.tile([128, 3, 256], mm_dt)
            nc.sync.dma_start(out=u2_t[64:128, :, :], in_=mm_view(u2_d[:]))
            bias_t = consts.tile([128, 2], F32)
            nc.sync.dma_start(out=bias_t, in_=bias_d[:])

            # ---- state (ping-pong) ----
            hz3 = hz_d[:].rearrange("p (a b) -> p a b", a=PH)
            hbufs = []
            for i in range(2):
                hb = state.tile([128, PH, PW], mm_dt, tag=f"hb{i}")
                nc.sync.dma_start(out=hb, in_=mm_view(hz3))
                hbufs.append(hb)
            cbufs = []
            for i in range(2):
                cb = state.tile([128, NPIX], F32, tag=f"cb{i}")
                cbufs.append(cb)
            # only c[t=0] needs zero-init; cb1 is fully written at t=0
            nc.sync.dma_start(out=cbufs[0][64:128, :], in_=cz_d[64:128, :])

            for t in range(T):
                hcur = hbufs[t % 2]
                hnxt = hbufs[(t + 1) % 2]
                ccur = cbufs[t % 2]
                cnxt = cbufs[(t + 1) % 2]

                xq_t = xqp.tile([96, PH, PW], mm_dt, tag="xq")
                nc.sync.dma_start(out=xq_t, in_=mm_view(xq_d[t]))

                for sb in range(SBLK):
                    r0 = SBH * sb
                    zA = psp.tile([128, NB], F32, tag="zA")
                    zB = psp.tile([128, NB], F32, tag="zB")
                    for m, z in ((0, zA), (1, zB)):
                        mc = slice(128 * m, 128 * m + 128)
                        for half in range(2):
                            rr = r0 + MMR * half
                            out = z[:, 512 * half:512 * half + 512]
                            for j in range(3):
                                nc.tensor.matmul(
                                    out, win_t[:, j, mc],
                                    xq_t[:, rr:rr + MMR, j:j + W],
                                    start=(j == 0), stop=False)
                            for j in range(3):
                                nc.tensor.matmul(
                                    out, u_t[:, j, mc],
                                    hcur[:, rr:rr + MMR, j:j + W],
                                    start=False, stop=False)
                            for j in range(3):
                                nc.tensor.matmul(
                                    out, u2_t[64:128, j, mc],
                                    hcur[64:128, rr + 1:rr + MMR + 1, j:j + W],
                                    start=False, stop=(j == 2))

                    blk = slice(NB * sb, NB * sb + NB)
                    # gates:  zA = [i; f] (both sigmoid), zB = [g; o]
                    pif = gp.tile([128, NB], F32, tag="pif")
                    nc.scalar.activation(out=pif, in_=zA[:], func=AF.Sigmoid,
                                         bias=bias_t[:, 0:1], scale=1.0)
                    S = gp.tile([128, NB], F32, tag="S")      # [g ; c]
                    nc.scalar.activation(out=S[0:64, :], in_=zB[0:64, :],
                                         func=AF.Tanh,
                                         bias=bias_t[0:64, 1:2], scale=1.0)
                    ot = gp.tile([128, NB], F32, tag="ot")    # [- ; o]
                    nc.scalar.activation(out=ot[64:128, :], in_=zB[64:128, :],
                                         func=AF.Sigmoid,
                                         bias=bias_t[64:128, 1:2], scale=1.0)
                    nc.vector.tensor_copy(S[64:128, :], ccur[64:128, blk])
                    R = gp.tile([128, NB], F32, tag="R")      # [i*g ; f*c]
                    nc.vector.tensor_mul(R, pif, S)
                    R2 = gp.tile([128, NB], F32, tag="R2")    # [- ; i*g]
                    nc.gpsimd.tensor_copy(out=R2[64:128, :], in_=R[0:64, :])
                    nc.vector.tensor_add(cnxt[64:128, blk], R2[64:128, :],
                                         R[64:128, :])
                    tct = gp.tile([128, NB], F32, tag="tct")  # [- ; tanh(c')]
                    nc.scalar.activation(out=tct[64:128, :],
                                         in_=cnxt[64:128, blk],
                                         func=AF.Tanh, scale=1.0)
                    hs = gp.tile([128, NB], F32, tag="hs")    # [- ; h]
                    nc.vector.tensor_mul(hs[64:128, :], ot[64:128, :],
                                         tct[64:128, :])
                    # replicate h into both row-shifted copies (cast to mm_dt)
                    hs3 = hs[64:128, :].rearrange("p (a b) -> p a b", a=SBH)
                    nc.gpsimd.tensor_copy(
                        out=hnxt[64:128, r0:r0 + SBH, 1:65], in_=hs3)
                    nc.gpsimd.tensor_copy(
                        out=hnxt[0:64, r0 + 1:r0 + SBH + 1, 1:65], in_=hs3)
                    nc.sync.dma_start(
                        out=y_d[t, :, NB * sb:NB * sb + NB],
                        in_=hs[64:128, :])
    nc.compile()
    return nc


def _prep_inputs(inputs, label_values, kernel, recurrent_kernel, bias):
    x = np.ascontiguousarray(np.asarray(inputs, dtype=np.float32))
    lv = np.asarray(label_values).astype(np.int64)
    k = np.asarray(kernel, dtype=np.float32)
    rk_all = np.asarray(recurrent_kernel, dtype=np.float32)
    bias_all = np.asarray(bias, dtype=np.float32)

    io_np = ml_dtypes.bfloat16 if MM_DT == "bf16" else np.float32

    x_cm = x.transpose(0, 1, 4, 2, 3)                      # (B, T, 32, 64, 64)
    pad = np.zeros((B, T, CIN, 68, PW), np.float32)
    pad[:, :, :, 1:65, 1:65] = x_cm
    xq = np.empty((B, T, 96, PH, PW), np.float32)
    for q in range(3):
        xq[:, :, 32 * q:32 * q + 32] = pad[:, :, :, q:q + PH, :]
    xq = xq.astype(io_np)

    w_in = np.empty((96, 3, 256), np.float32)
    for q in range(3):
        w_in[32 * q:32 * q + 32] = k[q].transpose(1, 0, 2)  # (32, 3, 256)
    w_in = w_in.astype(io_np)

    rk = rk_all[lv]                                         # (B, 3, 3, 64, 256)
    u = np.empty((B, 128, 3, 256), np.float32)
    for q in range(2):
        u[:, 64 * q:64 * q + 64] = rk[:, q].transpose(0, 2, 1, 3)
    u = u.astype(io_np)
    u2 = rk[:, 2].transpose(0, 2, 1, 3).astype(io_np)       # (B, 64, 3, 256)

    bias_b = bias_all[lv]                                   # (B, 256)
    bias2 = np.ascontiguousarray(
        bias_b.reshape(B, 2, 128).transpose(0, 2, 1))       # (B, 128, 2)

    hz = np.zeros((128, PH * PW), io_np)
    cz = np.zeros((128, NPIX), np.float32)
    in_maps = []
    for b in range(B):
        in_maps.append({
            "xq": np.ascontiguousarray(xq[b]),
            "w_in": w_in,
            "u": np.ascontiguousarray(u[b]),
            "u2": np.ascontiguousarray(u2[b]),
            "bias2": bias2[b],
            "hz": hz,
            "cz": cz,
        })
    return in_maps


def _run(inputs, label_values, kernel, recurrent_kernel, bias, **spmd_kwargs):
    if "nc" not in _CACHE:
        _CACHE["nc"] = _build()
    nc = _CACHE["nc"]
    in_maps = _prep_inputs(inputs, label_values, kernel, recurrent_kernel, bias)
    res = run_bass_kernel_spmd(nc, in_maps, list(range(B)), **spmd_kwargs)
    y = np.stack([res.results[b]["y"] for b in range(B)])   # (B, T, 64, 4096)
    out = y.reshape(B, T, F, H, W).transpose(0, 1, 3, 4, 2)
    return np.ascontiguousarray(out.astype(np.float32)), res


def kernel(inputs, label_values, kernel, recurrent_kernel, bias):
    out, _ = _run(inputs, label_values, kernel, recurrent_kernel, bias)
    return out


# revision 4
# speedup vs baseline: 1.9167x; 1.9071x over previous
"""ConvLSTM2D with per-sample expert routing (MoE) on 8 Trainium2 NeuronCores.

Data-parallel over batch: one sample per core. Everything below is hardcoded
for the problem shapes:
  inputs (8, 10, 64, 64, 32) fp32, label_values (8,) int32,
  kernel (3, 3, 32, 256), recurrent_kernel (4, 3, 3, 64, 256), bias (4, 256).

Per core, per timestep, the 3x3 convs are computed as 9 accumulating matmuls
into PSUM using row-shifted channel-stacked replicas of the padded images:
  - input conv:  x replicated 3x (row shifts 0,1,2) -> K=96, 3 matmuls (one
    per kernel column j), built host-side and streamed per step.
  - recurrent conv: h kept in SBUF as 2 row-shifted copies (shifts 0,1) on
    partitions 0-63 / 64-127 -> 3 matmuls with K=128 (rows dy=0,1) plus 3
    matmuls with K=64 on partitions 64-127 (row dy=2).
Gate order is the natural [i, f, g, o]: PSUM chunk A = [i; f], chunk B = [g; o].
Gate math stays fp32 (PSUM accumulation, ACT sigmoid/tanh with fused bias,
DVE products); matmul operands are MM_DT (bf16 by default, float32r option).
"""

import numpy as np
import ml_dtypes

import concourse.bacc as bacc
import concourse.mybir as mybir
import concourse.tile as tile
from concourse.bass_utils import run_bass_kernel_spmd

F32 = mybir.dt.float32
F32R = mybir.dt.float32r
BF16 = mybir.dt.bfloat16
AF = mybir.ActivationFunctionType

MM_DT = "bf16"          # "bf16" | "f32r"

B, T, H, W, CIN = 8, 10, 64, 64, 32
F = 64
NPIX = H * W                 # 4096
PH, PW = 66, 66              # padded image in SBUF
SBLK = 4                     # super blocks per step
SBH = 16                     # image rows per super block
NB = SBH * W                 # 1024 free elems per psum tile
MMR = 8                      # image rows per matmul (N = 512)

_CACHE = {}


def _build():
    mm_dt = BF16 if MM_DT == "bf16" else F32R
    io_dt = BF16 if MM_DT == "bf16" else F32   # dram dtype of matmul operands

    def mm_view(dram_ap):
        return dram_ap if MM_DT == "bf16" else dram_ap.bitcast(F32R)

    nc = bacc.Bacc("TRN2", target_bir_lowering=False, debug=True)

    xq_d = nc.declare_dram_parameter("xq", [T, 96, PH, PW], io_dt, isOutput=False)
    win_d = nc.declare_dram_parameter("w_in", [96, 3, 256], io_dt, isOutput=False)
    u_d = nc.declare_dram_parameter("u", [128, 3, 256], io_dt, isOutput=False)
    u2_d = nc.declare_dram_parameter("u2", [64, 3, 256], io_dt, isOutput=False)
    bias_d = nc.declare_dram_parameter("bias2", [128, 2], F32, isOutput=False)
    hz_d = nc.declare_dram_parameter("hz", [128, PH * PW], io_dt, isOutput=False)
    cz_d = nc.declare_dram_parameter("cz", [128, NPIX], F32, isOutput=False)
    y_d = nc.declare_dram_parameter("y", [T, F, NPIX], F32, isOutput=True)

    with tile.TileContext(nc) as tc:
        with tc.tile_pool(name="consts", bufs=1) as consts, \
             tc.tile_pool(name="state", bufs=1) as state, \
             tc.tile_pool(name="xqp", bufs=2) as xqp, \
             tc.tile_pool(name="psum", bufs=2, space="PSUM") as psp, \
             tc.tile_pool(name="gates", bufs=2) as gp:

            # ---- constants ----
            win_t = consts.tile([96, 3, 256], mm_dt)
            nc.sync.dma_start(out=win_t, in_=mm_view(win_d[:]))
            u_t = consts.tile([128, 3, 256], mm_dt)
            nc.sync.dma_start(out=u_t, in_=mm_view(u_d[:]))
            u2_t = consts.tile([128, 3, 256], mm_dt)
            nc.sync.dma_start(out=u2_t[64:128, :, :], in_=mm_view(u2_d[:]))
            bias_t = consts.tile([128, 2], F32)
            nc.sync.dma_start(out=bias_t, in_=bias_d[:])

            # ---- state (ping-pong) ----
            hz3 = hz_d[:].rearrange("p (a b) -> p a b", a=PH)
            hbufs = []
            for i in range(2):
                hb = state.tile([128, PH, PW], mm_dt, tag=f"hb{i}")
                nc.sync.dma_start(out=hb, in_=mm_view(hz3))
                hbufs.append(hb)
            cbufs = []
            for i in range(2):
                cb = state.tile([128, NPIX], F32, tag=f"cb{i}")
                cbufs.append(cb)
            # only c[t=0] needs zero-init; cb1 is fully written at t=0
            nc.sync.dma_start(out=cbufs[0][64:128, :], in_=cz_d[64:128, :])

            for t in range(T):
                hcur = hbufs[t % 2]
                hnxt = hbufs[(t + 1) % 2]
                ccur = cbufs[t % 2]
                cnxt = cbufs[(t + 1) % 2]

                xq_t = xqp.tile([96, PH, PW], mm_dt, tag="xq")
                nc.sync.dma_start(out=xq_t, in_=mm_view(xq_d[t]))

                for sb in range(SBLK):
                    r0 = SBH * sb
                    zA = psp.tile([128, NB], F32, tag="zA")
                    zB = psp.tile([128, NB], F32, tag="zB")
                    for m, z in ((0, zA), (1, zB)):
                        mc = slice(128 * m, 128 * m + 128)
                        for half in range(2):
                            rr = r0 + MMR * half
                            out = z[:, 512 * half:512 * half + 512]
                            for j in range(3):
                                nc.tensor.matmul(
                                    out, win_t[:, j, mc],
                                    xq_t[:, rr:rr + MMR, j:j + W],
                                    start=(j == 0), stop=False)
                            for j in range(3):
                                nc.tensor.matmul(
                                    out, u_t[:, j, mc],
                                    hcur[:, rr:rr + MMR, j:j + W],
                                    start=False, stop=False)
                            for j in range(3):
                                nc.tensor.matmul(
                                    out, u2_t[64:128, j, mc],
                                    hcur[64:128, rr + 1:rr + MMR + 1, j:j + W],
                                    start=False, stop=(j == 2))

                    blk = slice(NB * sb, NB * sb + NB)
                    # gates:  zA = [i; f] (both sigmoid), zB = [g; o]
                    pif = gp.tile([128, NB], F32, tag="pif")
                    nc.scalar.activation(out=pif, in_=zA[:], func=AF.Sigmoid,
                                         bias=bias_t[:, 0:1], scale=1.0)
                    S = gp.tile([128, NB], F32, tag="S")      # [g ; c]
                    nc.scalar.activation(out=S[0:64, :], in_=zB[0:64, :],
                                         func=AF.Tanh,
                                         bias=bias_t[0:64, 1:2], scale=1.0)
                    ot = gp.tile([128, NB], F32, tag="ot")    # [- ; o]
                    nc.scalar.activation(out=ot[64:128, :], in_=zB[64:128, :],
                                         func=AF.Sigmoid,
                                         bias=bias_t[64:128, 1:2], scale=1.0)
                    nc.vector.tensor_copy(S[64:128, :], ccur[64:128, blk])
                    R = gp.tile([128, NB], F32, tag="R")      # [i*g ; f*c]
                    nc.vector.tensor_mul(R, pif, S)
                    R2 = gp.tile([128, NB], F32, tag="R2")    # [- ; i*g]
                    nc.sync.dma_start(out=R2[64:128, :], in_=R[0:64, :])
                    nc.vector.tensor_add(cnxt[64:128, blk], R2[64:128, :],
                                         R[64:128, :])
                    tct = gp.tile([128, NB], F32, tag="tct")  # [- ; tanh(c')]
                    nc.scalar.activation(out=tct[64:128, :],
                                         in_=cnxt[64:128, blk],
                                         func=AF.Tanh, scale=1.0)
                    hs = gp.tile([128, NB], F32, tag="hs")    # [- ; h]
                    nc.vector.tensor_mul(hs[64:128, :], ot[64:128, :],
                                         tct[64:128, :])
                    # replicate h into both row-shifted copies (cast to mm_dt)
                    hs3 = hs[64:128, :].rearrange("p (a b) -> p a b", a=SBH)
                    nc.vector.tensor_copy(
                        hnxt[64:128, r0:r0 + SBH, 1:65], hs3)
                    nc.gpsimd.tensor_copy(
                        out=hnxt[0:64, r0 + 1:r0 + SBH + 1, 1:65], in_=hs3)
                    nc.sync.dma_start(
                        out=y_d[t, :, NB * sb:NB * sb + NB],
                        in_=hs[64:128, :])
    nc.compile()
    return nc


def _prep_inputs(inputs, label_values, kernel, recurrent_kernel, bias):
    x = np.ascontiguousarray(np.asarray(inputs, dtype=np.float32))
    lv = np.asarray(label_values).astype(np.int64)
    k = np.asarray(kernel, dtype=np.float32)
    rk_all = np.asarray(recurrent_kernel, dtype=np.float32)
    bias_all = np.asarray(bias, dtype=np.float32)

    io_np = ml_dtypes.bfloat16 if MM_DT == "bf16" else np.float32

    x_cm = x.transpose(0, 1, 4, 2, 3)                      # (B, T, 32, 64, 64)
    pad = np.zeros((B, T, CIN, 68, PW), np.float32)
    pad[:, :, :, 1:65, 1:65] = x_cm
    xq = np.empty((B, T, 96, PH, PW), np.float32)
    for q in range(3):
        xq[:, :, 32 * q:32 * q + 32] = pad[:, :, :, q:q + PH, :]
    xq = xq.astype(io_np)

    w_in = np.empty((96, 3, 256), np.float32)
    for q in range(3):
        w_in[32 * q:32 * q + 32] = k[q].transpose(1, 0, 2)  # (32, 3, 256)
    w_in = w_in.astype(io_np)

    rk = rk_all[lv]                                         # (B, 3, 3, 64, 256)
    u = np.empty((B, 128, 3, 256), np.float32)
    for q in range(2):
        u[:, 64 * q:64 * q + 64] = rk[:, q].transpose(0, 2, 1, 3)
    u = u.astype(io_np)
    u2 = rk[:, 2].transpose(0, 2, 1, 3).astype(io_np)       # (B, 64, 3, 256)

    bias_b = bias_all[lv]                                   # (B, 256)
    bias2 = np.ascontiguousarray(
        bias_b.reshape(B, 2, 128).transpose(0, 2, 1))       # (B, 128, 2)

    hz = np.zeros((128, PH * PW), io_np)
    cz = np.zeros((128, NPIX), np.float32)
    in_maps = []
    for b in range(B):
        in_maps.append({
            "xq": np.ascontiguousarray(xq[b]),
            "w_in": w_in,
            "u": np.ascontiguousarray(u[b]),
            "u2": np.ascontiguousarray(u2[b]),
            "bias2": bias2[b],
            "hz": hz,
            "cz": cz,
        })
    return in_maps


def _run(inputs, label_values, kernel, recurrent_kernel, bias, **spmd_kwargs):
    if "nc" not in _CACHE:
        _CACHE["nc"] = _build()
    nc = _CACHE["nc"]
    in_maps = _prep_inputs(inputs, label_values, kernel, recurrent_kernel, bias)
    res = run_bass_kernel_spmd(nc, in_maps, list(range(B)), **spmd_kwargs)
    y = np.stack([res.results[b]["y"] for b in range(B)])   # (B, T, 64, 4096)
    out = y.reshape(B, T, F, H, W).transpose(0, 1, 3, 4, 2)
    return np.ascontiguousarray(out.astype(np.float32)), res


def kernel(inputs, label_values, kernel, recurrent_kernel, bias):
    out, _ = _run(inputs, label_values, kernel, recurrent_kernel, bias)
    return out


# revision 7
# speedup vs baseline: 1.9523x; 1.0186x over previous
"""ConvLSTM2D with per-sample expert routing (MoE) on 8 Trainium2 NeuronCores.

Data-parallel over batch: one sample per core. Everything below is hardcoded
for the problem shapes:
  inputs (8, 10, 64, 64, 32) fp32, label_values (8,) int32,
  kernel (3, 3, 32, 256), recurrent_kernel (4, 3, 3, 64, 256), bias (4, 256).

Per core, per timestep, the 3x3 convs are computed as 9 accumulating matmuls
into PSUM using row-shifted channel-stacked replicas of the padded images:
  - input conv:  x replicated 3x (row shifts 0,1,2) -> K=96, 3 matmuls (one
    per kernel column j), built host-side and streamed per step.
  - recurrent conv: h kept in SBUF as 2 row-shifted copies (shifts 0,1) on
    partitions 0-63 / 64-127 -> 3 matmuls with K=128 (rows dy=0,1) plus 3
    matmuls with K=64 on partitions 64-127 (row dy=2).
Gate order is the natural [i, f, g, o]: PSUM chunk A = [i; f], chunk B = [g; o].
Gate math stays fp32 (PSUM accumulation, ACT sigmoid/tanh with fused bias,
DVE products); matmul operands are MM_DT (bf16 by default, float32r option).
"""

import numpy as np
import ml_dtypes

import concourse.bacc as bacc
import concourse.mybir as mybir
import concourse.tile as tile
from concourse.bass_utils import run_bass_kernel_spmd

F32 = mybir.dt.float32
F32R = mybir.dt.float32r
BF16 = mybir.dt.bfloat16
AF = mybir.ActivationFunctionType

MM_DT = "bf16"          # "bf16" | "f32r"

B, T, H, W, CIN = 8, 10, 64, 64, 32
F = 64
NPIX = H * W                 # 4096
PH, PW = 66, 66              # padded image in SBUF
SBLK = 4                     # super blocks per step
SBH = 16                     # image rows per super block
NB = SBH * W                 # 1024 free elems per psum tile
MMR = 8                      # image rows per matmul (N = 512)

_CACHE = {}


def _build():
    mm_dt = BF16 if MM_DT == "bf16" else F32R
    io_dt = BF16 if MM_DT == "bf16" else F32   # dram dtype of matmul operands

    def mm_view(dram_ap):
        return dram_ap if MM_DT == "bf16" else dram_ap.bitcast(F32R)

    nc = bacc.Bacc("TRN2", target_bir_lowering=False, debug=True)

    xq_d = nc.declare_dram_parameter("xq", [T, 96, PH, PW], io_dt, isOutput=False)
    win_d = nc.declare_dram_parameter("w_in", [96, 3, 256], io_dt, isOutput=False)
    u_d = nc.declare_dram_parameter("u", [128, 3, 256], io_dt, isOutput=False)
    u2_d = nc.declare_dram_parameter("u2", [64, 3, 256], io_dt, isOutput=False)
    bias_d = nc.declare_dram_parameter("bias2", [128, 2], F32, isOutput=False)
    hz_d = nc.declare_dram_parameter("hz", [128, PH * PW], io_dt, isOutput=False)
    cz_d = nc.declare_dram_parameter("cz", [128, NPIX], F32, isOutput=False)
    y_d = nc.declare_dram_parameter("y", [T, F, NPIX], io_dt, isOutput=True)

    with tile.TileContext(nc) as tc:
        with tc.tile_pool(name="consts", bufs=1) as consts, \
             tc.tile_pool(name="state", bufs=1) as state, \
             tc.tile_pool(name="xqp", bufs=2) as xqp, \
             tc.tile_pool(name="psum", bufs=2, space="PSUM") as psp, \
             tc.tile_pool(name="gates", bufs=2) as gp:

            # ---- constants ----
            win_t = consts.tile([96, 3, 256], mm_dt)
            nc.sync.dma_start(out=win_t, in_=mm_view(win_d[:]))
            u_t = consts.tile([128, 3, 256], mm_dt)
            nc.sync.dma_start(out=u_t, in_=mm_view(u_d[:]))
            u2_t = consts.tile([128, 3, 256], mm_dt)
            nc.sync.dma_start(out=u2_t[64:128, :, :], in_=mm_view(u2_d[:]))
            bias_t = consts.tile([128, 2], F32)
            nc.sync.dma_start(out=bias_t, in_=bias_d[:])

            # ---- state (ping-pong) ----
            hz3 = hz_d[:].rearrange("p (a b) -> p a b", a=PH)
            hbufs = []
            for i in range(2):
                hb = state.tile([128, PH, PW], mm_dt, tag=f"hb{i}")
                nc.sync.dma_start(out=hb, in_=mm_view(hz3))
                hbufs.append(hb)
            cbufs = []
            for i in range(2):
                cb = state.tile([128, NPIX], F32, tag=f"cb{i}")
                cbufs.append(cb)
            # only c[t=0] needs zero-init; cb1 is fully written at t=0
            nc.sync.dma_start(out=cbufs[0][64:128, :], in_=cz_d[64:128, :])

            for t in range(T):
                hcur = hbufs[t % 2]
                hnxt = hbufs[(t + 1) % 2]
                ccur = cbufs[t % 2]
                cnxt = cbufs[(t + 1) % 2]

                xq_t = xqp.tile([96, PH, PW], mm_dt, tag="xq")
                nc.sync.dma_start(out=xq_t, in_=mm_view(xq_d[t]))

                for sb in range(SBLK):
                    r0 = SBH * sb
                    zA = psp.tile([128, NB], F32, tag="zA")
                    zB = psp.tile([128, NB], F32, tag="zB")
                    for m, z in ((0, zA), (1, zB)):
                        mc = slice(128 * m, 128 * m + 128)
                        for half in range(2):
                            rr = r0 + MMR * half
                            out = z[:, 512 * half:512 * half + 512]
                            for j in range(3):
                                nc.tensor.matmul(
                                    out, win_t[:, j, mc],
                                    xq_t[:, rr:rr + MMR, j:j + W],
                                    start=(j == 0), stop=False)
                            for j in range(3):
                                nc.tensor.matmul(
                                    out, u_t[:, j, mc],
                                    hcur[:, rr:rr + MMR, j:j + W],
                                    start=False, stop=False)
                            for j in range(3):
                                nc.tensor.matmul(
                                    out, u2_t[64:128, j, mc],
                                    hcur[64:128, rr + 1:rr + MMR + 1, j:j + W],
                                    start=False, stop=(j == 2))

                    blk = slice(NB * sb, NB * sb + NB)
                    # gates:  zA = [i; f] (both sigmoid), zB = [g; o]
                    pif = gp.tile([128, NB], F32, tag="pif")
                    nc.scalar.activation(out=pif, in_=zA[:], func=AF.Sigmoid,
                                         bias=bias_t[:, 0:1], scale=1.0)
                    S = gp.tile([128, NB], F32, tag="S")      # [g ; c]
                    nc.scalar.activation(out=S[0:64, :], in_=zB[0:64, :],
                                         func=AF.Tanh,
                                         bias=bias_t[0:64, 1:2], scale=1.0)
                    ot = gp.tile([128, NB], F32, tag="ot")    # [- ; o]
                    nc.scalar.activation(out=ot[64:128, :], in_=zB[64:128, :],
                                         func=AF.Sigmoid,
                                         bias=bias_t[64:128, 1:2], scale=1.0)
                    nc.vector.tensor_copy(S[64:128, :], ccur[64:128, blk])
                    R = gp.tile([128, NB], F32, tag="R")      # [i*g ; f*c]
                    nc.vector.tensor_mul(R, pif, S)
                    R2 = gp.tile([128, NB], F32, tag="R2")    # [- ; i*g]
                    nc.sync.dma_start(out=R2[64:128, :], in_=R[0:64, :])
                    nc.vector.tensor_add(cnxt[64:128, blk], R2[64:128, :],
                                         R[64:128, :])
                    tct = gp.tile([128, NB], F32, tag="tct")  # [- ; tanh(c')]
                    nc.scalar.activation(out=tct[64:128, :],
                                         in_=cnxt[64:128, blk],
                                         func=AF.Tanh, scale=1.0)
                    # h = o * tanh(c') straight into the shifted copy (q=1),
                    # cast to mm_dt by the DVE write
                    nc.vector.tensor_mul(
                        hnxt[64:128, r0:r0 + SBH, 1:65],
                        ot[64:128, :].rearrange("p (a b) -> p a b", a=SBH),
                        tct[64:128, :].rearrange("p (a b) -> p a b", a=SBH))
                    # replicate into the unshifted copy (q=0), crossing lanes
                    nc.gpsimd.tensor_copy(
                        out=hnxt[0:64, r0 + 1:r0 + SBH + 1, 1:65],
                        in_=hnxt[64:128, r0:r0 + SBH, 1:65])
                    ysrc = hnxt[64:128, r0:r0 + SBH, 1:65]
                    if MM_DT != "bf16":
                        ysrc = ysrc.bitcast(F32)
                    nc.sync.dma_start(
                        out=y_d[t, :, NB * sb:NB * sb + NB], in_=ysrc)
    nc.compile()
    return nc


def _prep_inputs(inputs, label_values, kernel, recurrent_kernel, bias):
    x = np.ascontiguousarray(np.asarray(inputs, dtype=np.float32))
    lv = np.asarray(label_values).astype(np.int64)
    k = np.asarray(kernel, dtype=np.float32)
    rk_all = np.asarray(recurrent_kernel, dtype=np.float32)
    bias_all = np.asarray(bias, dtype=np.float32)

    io_np = ml_dtypes.bfloat16 if MM_DT == "bf16" else np.float32

    x_cm = x.transpose(0, 1, 4, 2, 3)                      # (B, T, 32, 64, 64)
    pad = np.zeros((B, T, CIN, 68, PW), np.float32)
    pad[:, :, :, 1:65, 1:65] = x_cm
    xq = np.empty((B, T, 96, PH, PW), np.float32)
    for q in range(3):
        xq[:, :, 32 * q:32 * q + 32] = pad[:, :, :, q:q + PH, :]
    xq = xq.astype(io_np)

    w_in = np.empty((96, 3, 256), np.float32)
    for q in range(3):
        w_in[32 * q:32 * q + 32] = k[q].transpose(1, 0, 2)  # (32, 3, 256)
    w_in = w_in.astype(io_np)

    rk = rk_all[lv]                                         # (B, 3, 3, 64, 256)
    u = np.empty((B, 128, 3, 256), np.float32)
    for q in range(2):
        u[:, 64 * q:64 * q + 64] = rk[:, q].transpose(0, 2, 1, 3)
    u = u.astype(io_np)
    u2 = rk[:, 2].transpose(0, 2, 1, 3).astype(io_np)       # (B, 64, 3, 256)

    bias_b = bias_all[lv]                                   # (B, 256)
    bias2 = np.ascontiguousarray(
        bias_b.reshape(B, 2, 128).transpose(0, 2, 1))       # (B, 128, 2)

    hz = np.zeros((128, PH * PW), io_np)
    cz = np.zeros((128, NPIX), np.float32)
    in_maps = []
    for b in range(B):
        in_maps.append({
            "xq": np.ascontiguousarray(xq[b]),
            "w_in": w_in,
            "u": np.ascontiguousarray(u[b]),
            "u2": np.ascontiguousarray(u2[b]),
            "bias2": bias2[b],
            "hz": hz,
            "cz": cz,
        })
    return in_maps


def _run(inputs, label_values, kernel, recurrent_kernel, bias, **spmd_kwargs):
    if "nc" not in _CACHE:
        _CACHE["nc"] = _build()
    nc = _CACHE["nc"]
    in_maps = _prep_inputs(inputs, label_values, kernel, recurrent_kernel, bias)
    res = run_bass_kernel_spmd(nc, in_maps, list(range(B)), **spmd_kwargs)
    y = np.stack([res.results[b]["y"] for b in range(B)])   # (B, T, 64, 4096)
    out = y.reshape(B, T, F, H, W).transpose(0, 1, 3, 4, 2)
    return np.ascontiguousarray(out.astype(np.float32)), res


def kernel(inputs, label_values, kernel, recurrent_kernel, bias):
    out, _ = _run(inputs, label_values, kernel, recurrent_kernel, bias)
    return out
